# revision 4
# baseline (speedup 1.0000x reference)
"""Edge-decoder (GNN link prediction) kernel for 8 Trainium2 NeuronCores.

Computes logits[e] = sum_d x[src[e], d] * x[tar[e], d] for 640K edges
(pos then neg), given node table x [100000, 128] f32.

Strategy: shard edges contiguously across the 8 cores (80000 each, padded
to 81920), x replicated. Per core, edges are processed in 40 chunks of
2048; each chunk issues 32 indirect DMA gathers (128 rows of 512B each,
one row index per SBUF partition - the only data-dependent addressing
primitive this runtime supports), then one fused elementwise multiply and
one blocked reduction on the vector engine, then an 8KB result writeback.
"""

import numpy as np

N_NODES = 100000
D = 128
E_TOTAL = 640000
N_CORES = 8
P = 128

E_CORE = E_TOTAL // N_CORES  # 80000
K = 16  # groups of 128 edges per chunk
CH = P * K  # 2048 edges per chunk
NCH = (E_CORE + CH - 1) // CH  # 40
E_PAD = NCH * CH  # 81920

_cached = {}


def _build(repeat=1):
    from concourse import bacc, mybir, tile
    from concourse.bass import IndirectOffsetOnAxis

    nc = bacc.Bacc(
        "TRN2", target_bir_lowering=False, debug=False, num_devices=N_CORES
    )
    x = nc.dram_tensor("x", [N_NODES, D], mybir.dt.float32, kind="ExternalInput").ap()
    idx = nc.dram_tensor(
        "edge_idx", [NCH, P, 2 * K], mybir.dt.int32, kind="ExternalInput"
    ).ap()
    logits = nc.dram_tensor(
        "logits", [NCH, P, K], mybir.dt.float32, kind="ExternalOutput"
    ).ap()

    with tile.TileContext(nc) as tc:
        with tc.tile_pool(name="idxp", bufs=3) as idxp, tc.tile_pool(
            name="rows", bufs=3
        ) as rowsp, tc.tile_pool(name="outp", bufs=3) as outp:
            for c in [c for _ in range(repeat) for c in range(NCH)]:
                idx_t = idxp.tile([P, 2 * K], mybir.dt.int32, tag="idx")
                nc.sync.dma_start(out=idx_t[:], in_=idx[c])
                src_rows = rowsp.tile([P, K * D], mybir.dt.float32, tag="src")
                tar_rows = rowsp.tile([P, K * D], mybir.dt.float32, tag="tar")
                for g in range(K):
                    nc.gpsimd.indirect_dma_start(
                        out=src_rows[:, g * D : (g + 1) * D],
                        out_offset=None,
                        in_=x,
                        in_offset=IndirectOffsetOnAxis(ap=idx_t[:, g : g + 1], axis=0),
                    )
                    nc.gpsimd.indirect_dma_start(
                        out=tar_rows[:, g * D : (g + 1) * D],
                        out_offset=None,
                        in_=x,
                        in_offset=IndirectOffsetOnAxis(
                            ap=idx_t[:, K + g : K + g + 1], axis=0
                        ),
                    )
                nc.vector.tensor_tensor(
                    out=src_rows[:],
                    in0=src_rows[:],
                    in1=tar_rows[:],
                    op=mybir.AluOpType.mult,
                )
                lg = outp.tile([P, K], mybir.dt.float32, tag="lg")
                nc.vector.tensor_reduce(
                    out=lg[:],
                    in_=src_rows[:].rearrange("p (g f) -> p g f", g=K),
                    axis=mybir.AxisListType.X,
                    op=mybir.AluOpType.add,
                )
                nc.sync.dma_start(out=logits[c], in_=lg[:])
    nc.compile()
    return nc


def _get_nc():
    if "nc" not in _cached:
        _cached["nc"] = _build()
    return _cached["nc"]


def build_repeated(repeat):
    """For test-harness delta timing: same program with the whole edge
    workload repeated `repeat` times (same outputs written each pass)."""
    return _build(repeat)


def _pack_core(src_c, tar_c):
    """src_c/tar_c: [E_PAD] int32 -> [NCH, P, 2K] with [c,p,g]=edge c*CH+g*128+p."""
    s = src_c.reshape(NCH, K, P).transpose(0, 2, 1)  # [NCH, P, K]
    t = tar_c.reshape(NCH, K, P).transpose(0, 2, 1)
    return np.ascontiguousarray(np.concatenate([s, t], axis=2))


def kernel(x, pos_edge_index, neg_edge_index):
    from concourse.bass_utils import run_bass_kernel_spmd

    x = np.ascontiguousarray(np.asarray(x, dtype=np.float32))
    src = np.concatenate(
        [np.asarray(pos_edge_index[0]), np.asarray(neg_edge_index[0])]
    ).astype(np.int32)
    tar = np.concatenate(
        [np.asarray(pos_edge_index[1]), np.asarray(neg_edge_index[1])]
    ).astype(np.int32)

    nc = _get_nc()
    in_maps = []
    for c in range(N_CORES):
        s = src[c * E_CORE : (c + 1) * E_CORE]
        t = tar[c * E_CORE : (c + 1) * E_CORE]
        s_pad = np.zeros(E_PAD, np.int32)
        t_pad = np.zeros(E_PAD, np.int32)
        s_pad[:E_CORE] = s
        t_pad[:E_CORE] = t
        in_maps.append({"x": x, "edge_idx": _pack_core(s_pad, t_pad)})

    res = run_bass_kernel_spmd(nc, in_maps, core_ids=list(range(N_CORES)))
    out = np.empty((E_TOTAL, 1), np.float32)
    for c in range(N_CORES):
        lg = res.results[c]["logits"]  # [NCH, P, K]
        flat = lg.transpose(0, 2, 1).reshape(E_PAD)  # edge c*CH+g*128+p order
        out[c * E_CORE : (c + 1) * E_CORE, 0] = flat[:E_CORE]
    return out


# revision 6
# speedup vs baseline: 12.8785x; 12.8785x over previous
"""Edge-decoder (GNN link prediction) kernel for 8 Trainium2 NeuronCores.

Computes logits[e] = sum_d x[src[e], d] * x[tar[e], d] for 640K edges
(pos then neg), node table x [100000, 128] f32.

Strategy (edges sharded contiguously across 8 cores, x replicated):
per core, edges are sorted by src id on the host and processed in groups
of 128. Each group's src rows all lie in a row window [base_g, base_g+W)
(W covers the max group span); the window is DMA-loaded with a
register-driven dynamic DRAM offset and the 128 src rows are selected
from it on the tensor engine with one-hot matrices (built by comparing
host-provided row-id columns against PE-replicated edge ids), accumulated
in PSUM over the W/128 sub-windows. The tar rows use the [P,1]-indexed
indirect DMA (the only data-dependent addressing primitive available
on this runtime - 128 rows per Pool-engine instruction), halving the
number of those serialized instructions vs gathering both sides.
Product + free-dim reduction give a logit column per group; the host
inverse-permutes the sorted logits back to edge order.
"""

import numpy as np

N_NODES = 100000
D = 128
E_TOTAL = 640000
N_CORES = 8
P = 128
SGW = 16  # groups per supergroup (idx/output tile width)

E_CORE = E_TOTAL // N_CORES  # 80000
NG = E_CORE // P  # 625 groups
NSG = (NG + SGW - 1) // SGW  # 40 supergroups

_cached = {}


def _build(w_cap):
    from concourse import bacc, mybir, tile
    from concourse.bass import IndirectOffsetOnAxis, ds

    kw = w_cap // P
    nc = bacc.Bacc(
        "TRN2", target_bir_lowering=False, debug=False, num_devices=N_CORES
    )
    x = nc.dram_tensor("x", [N_NODES, D], mybir.dt.float32, kind="ExternalInput").ap()
    bases = nc.dram_tensor(
        "bases", [1, NG], mybir.dt.int32, kind="ExternalInput"
    ).ap()
    row_ids = nc.dram_tensor(
        "row_ids", [P, kw * NG], mybir.dt.float32, kind="ExternalInput"
    ).ap()
    src_f = nc.dram_tensor(
        "src_ids_f", [NSG, SGW * P], mybir.dt.float32, kind="ExternalInput"
    ).ap()
    tar_i = nc.dram_tensor(
        "tar_idx", [NSG, P, SGW], mybir.dt.int32, kind="ExternalInput"
    ).ap()
    logits = nc.dram_tensor(
        "logits", [NSG, P, SGW], mybir.dt.float32, kind="ExternalOutput"
    ).ap()

    with tile.TileContext(nc) as tc:
        with tc.tile_pool(name="xw", bufs=6) as xwp, tc.tile_pool(
            name="sel", bufs=8
        ) as selp, tc.tile_pool(name="repl", bufs=4, space="PSUM") as replp, tc.tile_pool(
            name="sacc", bufs=4, space="PSUM"
        ) as saccp, tc.tile_pool(name="trows", bufs=12) as trowsp, tc.tile_pool(
            name="io", bufs=3
        ) as iop, tc.tile_pool(name="misc", bufs=1) as miscp:
            ones_t = miscp.tile([1, P], mybir.dt.float32)
            nc.vector.memset(ones_t[:], 1.0)
            bases_t = miscp.tile([1, NG], mybir.dt.int32)
            nc.sync.dma_start(out=bases_t[:], in_=bases)
            rid_t = miscp.tile([P, kw * NG], mybir.dt.float32)
            nc.sync.dma_start(out=rid_t[:], in_=row_ids)
            scratch = miscp.tile([P, D], mybir.dt.float32)

            for sg in range(NSG):
                ids_t = iop.tile([1, SGW * P], mybir.dt.float32, tag="ids")
                nc.sync.dma_start(out=ids_t[:], in_=src_f[sg])
                tidx_t = iop.tile([P, SGW], mybir.dt.int32, tag="tidx")
                nc.sync.dma_start(out=tidx_t[:], in_=tar_i[sg])
                lg_t = iop.tile([P, SGW], mybir.dt.float32, tag="lg")
                g0 = sg * SGW
                g1 = min(g0 + SGW, NG)
                if g1 - g0 < SGW:
                    nc.vector.memset(lg_t[:], 0.0)
                for g in range(g0, g1):
                    j = g - g0
                    # window load on the ACT HWDGE queue so the Pool engine's
                    # upstream (tidx loads on SP) never queues behind it
                    base = nc.values_load(
                        bases_t[0:1, g : g + 1],
                        engines=(mybir.EngineType.Activation,),
                        min_val=0,
                        max_val=N_NODES - w_cap,
                        skip_runtime_bounds_check=True,
                    )
                    xw = xwp.tile([P, kw, D], mybir.dt.float32, tag="xw")
                    nc.scalar.dma_start(
                        out=xw[:],
                        in_=x[ds(base, w_cap), :].rearrange("(k p) f -> p k f", p=P),
                    )
                    rt = replp.tile([P, P], mybir.dt.float32, tag="repl")
                    nc.tensor.matmul(
                        out=rt[:],
                        lhsT=ones_t[:],
                        rhs=ids_t[:, j * P : (j + 1) * P],
                        start=True,
                        stop=True,
                    )
                    sa = saccp.tile([P, D], mybir.dt.float32, tag="sacc")
                    for k in range(kw):
                        sel = selp.tile([P, P], mybir.dt.float32, tag="sel")
                        nc.vector.tensor_tensor(
                            out=sel[:],
                            in0=rid_t[
                                :, g * kw + k : g * kw + k + 1
                            ].to_broadcast((P, P)),
                            in1=rt[:],
                            op=mybir.AluOpType.is_equal,
                        )
                        nc.tensor.matmul(
                            out=sa[:],
                            lhsT=sel[:],
                            rhs=xw[:, k, :],
                            start=(k == 0),
                            stop=(k == kw - 1),
                        )
                    trow = trowsp.tile([P, D], mybir.dt.float32, tag="trows")
                    nc.gpsimd.indirect_dma_start(
                        out=trow[:],
                        out_offset=None,
                        in_=x,
                        in_offset=IndirectOffsetOnAxis(
                            ap=tidx_t[:, j : j + 1], axis=0
                        ),
                    )
                    nc.vector.tensor_tensor(
                        out=trow[:],
                        in0=trow[:],
                        in1=sa[:],
                        op=mybir.AluOpType.mult,
                    )
                    nc.scalar.activation(
                        out=scratch[:],
                        in_=trow[:],
                        func=mybir.ActivationFunctionType.Copy,
                        accum_out=lg_t[:, j : j + 1],
                    )
                nc.sync.dma_start(out=logits[sg], in_=lg_t[:])
    nc.compile()
    return nc


def _get_nc(w_cap):
    if w_cap not in _cached:
        _cached[w_cap] = _build(w_cap)
    return _cached[w_cap]


def _host_prepare(x, src, tar, w_cap):
    """Per-core sort/pack. Returns (in_maps, perms, max_span)."""
    kw = w_cap // P
    xc = np.ascontiguousarray(np.asarray(x, np.float32))
    in_maps, perms = [], []
    max_span = 0
    for c in range(N_CORES):
        s = src[c * E_CORE : (c + 1) * E_CORE]
        t = tar[c * E_CORE : (c + 1) * E_CORE]
        perm = np.argsort(s, kind="stable")
        s_s = s[perm].astype(np.int64)
        t_s = t[perm]
        perms.append(perm)

        blk = s_s.reshape(NG, P)
        span = int((blk[:, -1] - blk[:, 0] + 1).max())
        max_span = max(max_span, span)
        base = np.minimum(blk[:, 0], N_NODES - w_cap).astype(np.int32)

        row_ids = (
            base[None, :, None].astype(np.float32)
            + (np.arange(kw, dtype=np.float32) * P)[None, None, :]
            + np.arange(P, dtype=np.float32)[:, None, None]
        ).reshape(P, NG * kw)

        pad_g = NSG * SGW - NG
        s_pad = np.concatenate([s_s, np.zeros(pad_g * P, s_s.dtype)])
        t_pad = np.concatenate([t_s, np.zeros(pad_g * P, t_s.dtype)])
        in_maps.append(
            {
                "x": xc,
                "bases": base[None, :],
                "row_ids": np.ascontiguousarray(row_ids),
                "src_ids_f": np.ascontiguousarray(
                    s_pad.astype(np.float32).reshape(NSG, SGW * P)
                ),
                "tar_idx": np.ascontiguousarray(
                    t_pad.reshape(NSG, SGW, P).transpose(0, 2, 1).astype(np.int32)
                ),
            }
        )
    return in_maps, perms, max_span


def kernel(x, pos_edge_index, neg_edge_index):
    from concourse.bass_utils import run_bass_kernel_spmd

    src = np.concatenate(
        [np.asarray(pos_edge_index[0]), np.asarray(neg_edge_index[0])]
    ).astype(np.int32)
    tar = np.concatenate(
        [np.asarray(pos_edge_index[1]), np.asarray(neg_edge_index[1])]
    ).astype(np.int32)

    w_cap = 256
    in_maps, perms, max_span = _host_prepare(x, src, tar, w_cap)
    if max_span > w_cap:
        w_cap = ((max_span + P - 1) // P) * P
        in_maps, perms, max_span = _host_prepare(x, src, tar, w_cap)

    nc = _get_nc(w_cap)
    res = run_bass_kernel_spmd(nc, in_maps, core_ids=list(range(N_CORES)))

    out = np.empty((E_TOTAL, 1), np.float32)
    for c in range(N_CORES):
        lg = res.results[c]["logits"]  # [NSG, P, SGW]
        flat = lg.transpose(0, 2, 1).reshape(-1)[: NG * P]
        oc = np.empty(E_CORE, np.float32)
        oc[perms[c]] = flat
        out[c * E_CORE : (c + 1) * E_CORE, 0] = oc
    return out
